# revision 7
# baseline (speedup 1.0000x reference)
"""BertSelfAttention Trainium2 Bass kernel.

Full (unsharded) inputs in, full output out. Internally shards across 8
NeuronCores as (batch b, head-group g): core c handles batch c//2 and
heads [6*(c%2), 6*(c%2)+6) of the 12 heads.

Per-core program (Tile framework):
  A) load hs[b], W/bias slices, mask[b]; PE-transpose to put the
     contraction dim on partitions (hsT [d,q], WT [d,out], maskT [k,1]).
  B) QT/KT [128=2 heads, 2048] via matmul; V [k, dh] directly (bias via
     rank-1 ones-row matmul); ones column appended per head for row-sums.
  C) per head, per q-chunk of 1024: flash-style loop over 16 k-tiles:
     scoresT [k-tile, q-chunk] in PSUM -> one ACT instruction does
     exp(0.125*s + mask_k) (scale folds 1/sqrt(64), per-partition bias
     folds the additive attention mask) -> probsT bf16 -> PV accumulates
     ctxT [65, q-chunk] in PSUM (row 64 = softmax denominator).
     Tail: PE-transpose ctxT -> [q, 65], DVE reciprocal + scale, DMA out.
"""

import os
import sys

sys.path.insert(0, "/opt/trn_rl_repo")

import numpy as np

B, S, D = 4, 2048, 768
H, DH = 12, 64
NCORES = 8
HPC = 6          # heads per core
GSZ = HPC * DH   # 384 output dims per core
P = 128
ND = D // P      # 6 d-tiles
NT = S // P      # 16 k-tiles
QC = 1024        # q-chunk
MMN = 512        # matmul free dim per instruction

_cache = {}


def _build(mm_dt_name: str):
    if mm_dt_name in _cache:
        return _cache[mm_dt_name]

    import concourse.bass as bass
    import concourse.bacc as bacc
    import concourse.mybir as mybir
    from concourse import tile
    from concourse.masks import make_identity

    f32 = mybir.dt.float32
    mm_dt = getattr(mybir.dt, mm_dt_name)
    AF = mybir.ActivationFunctionType

    nc = bacc.Bacc("TRN2", target_bir_lowering=False, debug=False,
                   num_devices=NCORES)

    hs_d = nc.dram_tensor("hs", [S, D], f32, kind="ExternalInput")
    w_d = {p: nc.dram_tensor(f"w{p}", [GSZ, D], f32, kind="ExternalInput")
           for p in "qkv"}
    bias_d = nc.dram_tensor("bias", [3, GSZ], f32, kind="ExternalInput")
    mask_d = nc.dram_tensor("mask", [NT, P], f32, kind="ExternalInput")
    out_d = nc.dram_tensor("out", [S, GSZ], f32, kind="ExternalOutput")

    with tile.TileContext(nc) as tc:
        with tc.tile_pool(name="const", bufs=1) as const_pool, \
             tc.tile_pool(name="persist", bufs=1) as pers:

            ident = const_pool.tile([P, P], f32)
            make_identity(nc, ident[:])
            ident_mm = const_pool.tile([P, P], mm_dt)
            make_identity(nc, ident_mm[:])

            # ---- persistent SBUF tensors ----
            hsT = pers.tile([P, ND, S], mm_dt, tag="hsT")       # [d%128, dtile, q]
            wT = {p: pers.tile([P, ND, GSZ], mm_dt, tag=f"wT{p}", name=f"wT{p}")
                  for p in "qkv"}
            maskT = pers.tile([P, NT], f32, tag="maskT")        # [k%128, ktile]
            biasT = pers.tile([P, 6], f32, tag="biasT")         # [dim%128, pair*2+proj(q,k)]
            qT = pers.tile([P, 3, S], mm_dt, tag="qT")          # [2*dh, pair, q]
            kT = pers.tile([P, 3, S], mm_dt, tag="kT")
            vsb = pers.tile([P, NT, HPC * (DH + 1)], mm_dt, tag="vsb")
            bvrow = pers.tile([1, GSZ], mm_dt, tag="bvrow")
            onesrow = pers.tile([1, P], mm_dt, tag="onesrow")

            nc.vector.memset(vsb[:], 1.0)     # ones columns; v dims overwritten
            nc.vector.memset(onesrow[:], 1.0)

            # ================= Phase A: loads + transposes =================
            with tc.tile_pool(name="stage", bufs=3) as stage, \
                 tc.tile_pool(name="trps", bufs=2, space="PSUM") as trps:

                # mask [NT, P] -> maskT [P, NT]
                mstage = stage.tile([NT, P], f32, tag="mstage")
                nc.sync.dma_start(mstage[:], mask_d[:])
                mps = trps.tile([P, NT], f32, tag="tr")
                nc.tensor.transpose(mps[:], mstage[:], ident[:NT, :NT])
                nc.vector.tensor_copy(maskT[:], mps[:])

                # bias [3, GSZ] -> biasT [P, proj(q,k), pair]; bv -> bvrow
                bstage = stage.tile([3, GSZ], f32, tag="bstage")
                nc.sync.dma_start(bstage[:], bias_d[:])
                for pp in range(3):
                    bps = trps.tile([P, 3], f32, tag="tr")
                    nc.tensor.transpose(bps[:], bstage[:, pp * P:(pp + 1) * P],
                                        ident[:3, :3])
                    nc.vector.tensor_copy(biasT[:, pp * 2:pp * 2 + 2], bps[:, 0:2])
                bvstage = stage.tile([1, GSZ], f32, tag="bvstage")
                nc.sync.dma_start(bvstage[:], bias_d[2:3, :])
                nc.vector.tensor_copy(bvrow[0:1, :], bvstage[0:1, :])

                # hs -> hsT (cast to mm_dt on eviction)
                for t in range(NT):
                    hstage = stage.tile([P, D], f32, tag="hstage")
                    nc.sync.dma_start(hstage[:], hs_d[t * P:(t + 1) * P, :])
                    for d in range(ND):
                        ps = trps.tile([P, P], f32, tag="tr")
                        nc.tensor.transpose(ps[:], hstage[:, d * P:(d + 1) * P],
                                            ident[:])
                        nc.vector.tensor_copy(hsT[:, d, t * P:(t + 1) * P], ps[:])

                # W slices -> wT
                for p in "qkv":
                    for r in range(GSZ // P):  # 3 row-tiles of 128 outdims
                        wstage = stage.tile([P, D], f32, tag="wstage")
                        nc.sync.dma_start(wstage[:], w_d[p][r * P:(r + 1) * P, :])
                        for d in range(ND):
                            ps = trps.tile([P, P], f32, tag="tr")
                            nc.tensor.transpose(ps[:],
                                                wstage[:, d * P:(d + 1) * P],
                                                ident[:])
                            nc.vector.tensor_copy(
                                wT[p][:, d, r * P:(r + 1) * P], ps[:])

            # ================= Phase B: QKV projections =================
            with tc.tile_pool(name="qkps", bufs=2, space="PSUM") as qkps, \
                 tc.tile_pool(name="vps", bufs=2, space="PSUM") as vps:

                # V [k, dh] per k-tile (+ bias via rank-1 ones x bv)
                for t in range(NT):
                    ps = vps.tile([P, GSZ], f32, tag="vps")
                    for d in range(ND):
                        nc.tensor.matmul(ps[:], hsT[:, d, t * P:(t + 1) * P],
                                         wT["v"][:, d, :],
                                         start=(d == 0), stop=False)
                    nc.tensor.matmul(ps[:], onesrow[0:1, :],
                                     bvrow[0:1, :], start=False, stop=True)
                    nc.vector.tensor_copy(
                        vsb[:, t, :].rearrange("p (h c) -> p h c", c=DH + 1)[:, :, 0:DH],
                        ps[:].rearrange("p (h c) -> p h c", c=DH))

                # QT / KT per head-pair
                for pp in range(3):
                    for pi, (pname, dst) in enumerate((("q", qT), ("k", kT))):
                        for half in range(2):
                            ps = qkps.tile([P, QC], f32, tag="qkps")
                            for d in range(ND):
                                for n in range(QC // MMN):
                                    o = half * QC + n * MMN
                                    nc.tensor.matmul(
                                        ps[:, n * MMN:(n + 1) * MMN],
                                        wT[pname][:, d, pp * P:(pp + 1) * P],
                                        hsT[:, d, o:o + MMN],
                                        start=(d == 0), stop=(d == ND - 1))
                            nc.vector.tensor_scalar_add(
                                dst[:, pp, half * QC:(half + 1) * QC],
                                ps[:], biasT[:, pp * 2 + pi:pp * 2 + pi + 1])

            # ================= Phase C: attention per head =================
            with tc.tile_pool(name="saps", bufs=2, space="PSUM") as saps, \
                 tc.tile_pool(name="ctxps", bufs=1, space="PSUM") as ctxps, \
                 tc.tile_pool(name="tpps", bufs=2, space="PSUM") as tpps, \
                 tc.tile_pool(name="probs", bufs=3) as probs_pool, \
                 tc.tile_pool(name="tailsb", bufs=2) as tailsb, \
                 tc.tile_pool(name="outsb", bufs=3) as outsb:

                for h in range(HPC):
                    pp, base = h // 2, (h % 2) * DH
                    kT_h = kT[base:base + DH, pp, :]
                    qT_h = qT[base:base + DH, pp, :]
                    for qc in range(S // QC):
                        ctx = ctxps.tile([DH + 1, QC], f32, tag="ctx")
                        for t in range(NT):
                            sa = saps.tile([P, QC], f32, tag="sa")
                            pr = probs_pool.tile([P, QC], mm_dt, tag="pr")
                            for n in range(QC // MMN):
                                nc.tensor.matmul(
                                    sa[:, n * MMN:(n + 1) * MMN],
                                    kT_h[:, t * P:(t + 1) * P],
                                    qT_h[:, qc * QC + n * MMN: qc * QC + (n + 1) * MMN],
                                    start=True, stop=True)
                            nc.scalar.activation(pr[:], sa[:], AF.Exp,
                                                 bias=maskT[:, t:t + 1],
                                                 scale=0.125)
                            for n in range(QC // MMN):
                                nc.tensor.matmul(
                                    ctx[:, n * MMN:(n + 1) * MMN],
                                    vsb[:, t, h * (DH + 1):(h + 1) * (DH + 1)],
                                    pr[:, n * MMN:(n + 1) * MMN],
                                    start=(t == 0), stop=(t == NT - 1))
                        # tail: normalize + transpose + store
                        ctxu = tailsb.tile([DH + 1, QC], f32, tag="ctxu")
                        nc.vector.tensor_copy(ctxu[:], ctx[:])
                        for s2 in range(QC // P):
                            tp = tpps.tile([P, DH + 1], f32, tag="tp")
                            nc.tensor.transpose(
                                tp[:], ctxu[:, s2 * P:(s2 + 1) * P],
                                ident[:DH + 1, :DH + 1])
                            rcp = outsb.tile([P, 1], f32, tag="rcp")
                            nc.vector.reciprocal(rcp[:], tp[:, DH:DH + 1])
                            ot = outsb.tile([P, DH], f32, tag="ot")
                            nc.vector.tensor_scalar_mul(ot[:], tp[:, 0:DH], rcp[:])
                            q0 = qc * QC + s2 * P
                            nc.sync.dma_start(
                                out_d[q0:q0 + P, h * DH:(h + 1) * DH], ot[:])

    nc.compile()
    _cache[mm_dt_name] = nc
    return nc


def _in_maps(hidden_states, attention_mask, Wq, bq, Wk, bk, Wv, bv):
    maps = []
    for c in range(NCORES):
        b, g = c // 2, c % 2
        sl = slice(g * GSZ, (g + 1) * GSZ)
        maps.append({
            "hs": np.ascontiguousarray(hidden_states[b], dtype=np.float32),
            "wq": np.ascontiguousarray(Wq[sl], dtype=np.float32),
            "wk": np.ascontiguousarray(Wk[sl], dtype=np.float32),
            "wv": np.ascontiguousarray(Wv[sl], dtype=np.float32),
            "bias": np.ascontiguousarray(
                np.stack([bq[sl], bk[sl], bv[sl]]), dtype=np.float32),
            "mask": np.ascontiguousarray(
                attention_mask[b].reshape(NT, P), dtype=np.float32),
        })
    return maps


def kernel(hidden_states, attention_mask, Wq, bq, Wk, bk, Wv, bv,
           _trace=False, _tmpdir=None):
    from concourse.bass_utils import run_bass_kernel_spmd

    nc = _build(os.environ.get("BERT_MM_DT", "bfloat16"))
    maps = _in_maps(np.asarray(hidden_states), np.asarray(attention_mask),
                    np.asarray(Wq), np.asarray(bq), np.asarray(Wk),
                    np.asarray(bk), np.asarray(Wv), np.asarray(bv))
    res = run_bass_kernel_spmd(nc, maps, core_ids=list(range(NCORES)),
                               trace=_trace, tmpdir=_tmpdir)
    out = np.empty((B, S, D), dtype=np.float32)
    for c in range(NCORES):
        b, g = c // 2, c % 2
        out[b, :, g * GSZ:(g + 1) * GSZ] = res.results[c]["out"]
    kernel.last_results = res
    return out
